# revision 9
# baseline (speedup 1.0000x reference)
"""ACE loss kernel for Trainium2, data-parallel over batch across 8 NeuronCores.

Math (matches the reference exactly):
  p[b,c]   = mean_t softmax(preds[t,b,:])[c]
  counts   = per-row histogram of trans_targets (blank bin overwritten with
             T - #positive-valid labels); note sum_c counts[b,c] == T.
  loss     = -sum_{b,c} log(p[b,c]) * counts[b,c] / (B*T)

Device work per core (B_local = 16 batch rows, 64 MB of preds):
  One streaming pass over preds as 1MiB chunk DMAs ([128 partitions =
  (8 t x 16 b), 2048 classes] each; the final row-tile tapers to
  [2048,2048,2048,1024,512,512] so only a 512-col exp trails the last DMA).
  Each chunk lands in its OWN tile so its ACT exp gates only on its own
  DMA; row-sum partials run on the otherwise-idle DVE (bf16 2x reduce),
  keeping ACT's (N+352)/1.2ns exp stream just below the 432 GB/s DMA rate
  with no accumulator-read overhead -- in fast reps ACT otherwise overhangs
  the stream end by ~6us.
    ACT: E = exp(x) (bf16) per chunk
    DVE: rs[:,c] = rowsum(E chunk); w = 1/sum(rs); lhsT = mask32 * w
    PE : per 512-class chunk j, psum[32*(j//4)+0:32, (j%4)*512 ...] +=
         lhsT.T @ E   (PSUM accumulates the whole T reduction; the 16 zero
         lhsT columns write exact 0.0 rows, so all 128 psum partitions are
         deterministic and the epilogue can run full-width)
  Epilogue (pipelined bank-by-bank with the last tile's matmuls):
    4x ACT Ln(psum/T + 1e-30) -> bf16 over [128, 512]  (1e-30 keeps the
    zero rows finite: Ln -> -69, then * count 0 -> 0), fused DVE 16-bit
    multiply-by-counts (bf16 counts, exact for integers) + row-sum per
    bank (last bank in halves to pipeline ACT/DVE), ones-matmul collapse
    to five scalars, single 20-byte output DMA from the Sync queue.
Host: tiny histogram of trans_targets, final sum of the 8 per-core outputs.
"""

import sys

sys.path.insert(0, "/opt/trn_rl_repo")

import numpy as np
import ml_dtypes

T, B, C, L = 128, 128, 8192, 50
NCORES = 8
BL = B // NCORES          # 16 batch rows per core
NCH = C // 512            # 16 class chunks of 512
NT = (T * BL) // 128      # 16 row-tiles of 128 (t,b) pairs
BLANK = 0
CK = 2048                 # stream chunk columns (1 MiB)
NCK = C // CK             # 4 chunks per row-tile
# Final row-tile taper (512-aligned so every matmul rhs stays in one tile).
PIECES = (2048, 2048, 2048, 1024, 512, 512)

_CACHE = {}


def _build_nc():
    from concourse import bacc, mybir
    import concourse.tile as tile

    f32 = mybir.dt.float32
    bf16 = mybir.dt.bfloat16
    AF = mybir.ActivationFunctionType
    MUL = mybir.AluOpType.mult
    ADD = mybir.AluOpType.add
    AX = mybir.AxisListType.X

    nc = bacc.Bacc("TRN2", target_bir_lowering=False, debug=False)
    preds = nc.dram_tensor("preds", [T * BL, C], f32, kind="ExternalInput")
    mask = nc.dram_tensor("mask", [128, 32], f32, kind="ExternalInput")
    counts2 = nc.dram_tensor("counts2", [128, 2048], bf16, kind="ExternalInput")
    out = nc.dram_tensor("out", [1, 5], f32, kind="ExternalOutput")

    with tile.TileContext(nc) as tc:
        with tc.tile_pool(name="xp", bufs=3) as xp, \
             tc.tile_pool(name="xtp", bufs=1) as xtp, \
             tc.tile_pool(name="ep", bufs=3) as ep, \
             tc.tile_pool(name="etp", bufs=1) as etp, \
             tc.tile_pool(name="sm", bufs=2) as sm, \
             tc.tile_pool(name="fin", bufs=1) as fin, \
             tc.tile_pool(name="pp", bufs=1, space="PSUM") as pp:
            # Preload the ACT table set that holds BOTH Exp and Ln
            # (natural_log_exp_and_others, id 6) so no table switch lands on
            # the critical tail between the last exp and the Ln epilogue.
            nc.scalar.add_instruction(mybir.InstLoadActFuncSet(
                name=nc.get_next_instruction_name(), ins=[], outs=[],
                act_func_set_id=6))

            # p lives in psum banks 0-3: chunk j at partitions
            # [32*(j//4), +32) (rows 16-31 exact zeros), bank j % 4.
            # One tile PER BANK: the Tile framework tracks hazards at tile
            # granularity, so a shared tile serializes bank i+1's matmuls
            # behind bank i's Ln read (WAR). Separate tiles pipeline.
            psum_b = [pp.tile([128, 512], f32, tag=f"psum{i}", name=f"psum{i}")
                      for i in range(4)]
            # Bank 4: final ones-matmul collapse target.
            psum_f = pp.tile([128, 512], f32, tag="psumf", name="psum_f")

            # Small inputs + epilogue tiles (memset runs early, off the
            # DMA-critical path).
            mask_sb = fin.tile([128, 32], f32, tag="mask")
            cnt_sb = fin.tile([128, 2048], bf16, tag="cnt")
            # bf16 log tiles: halves DVE time in the count-multiply STT
            # (16-bit 2x mode); bf16 abs err ~0.03 on real rows -> ~0.1%
            # loss error, far inside tolerance.
            logt = [fin.tile([128, 512], bf16, tag=f"logt{i}", name=f"logt{i}")
                    for i in range(4)]
            prod = [fin.tile([128, 512], bf16, tag=f"prod{i}", name=f"prod{i}")
                    for i in range(4)]
            ones = fin.tile([128, 1], f32, tag="ones")
            lnbias = fin.tile([128, 1], f32, tag="lnbias")
            red5 = fin.tile([128, 5], f32, tag="red5")

            for k in range(NT):
                last = k == NT - 1
                rows = preds.ap()[k * 128:(k + 1) * 128, :]
                pieces = PIECES if last else (CK,) * NCK
                # One tile per chunk: exp/reduce gate on their own DMA.
                xc, ec = [], []
                off = 0
                for q, pc in enumerate(pieces):
                    if pc == CK and q < NCK:
                        xt = xp.tile([128, CK], f32, tag=f"xc{q}",
                                     name=f"xc{q}_{k}")
                        et = ep.tile([128, CK], bf16, tag=f"ec{q}",
                                     name=f"ec{q}_{k}")
                    else:
                        xt = xtp.tile([128, pc], f32, tag=f"xt{q}",
                                      name=f"xt{q}")
                        et = etp.tile([128, pc], bf16, tag=f"et{q}",
                                      name=f"et{q}")
                    nc.sync.dma_start(xt[:], rows[:, off:off + pc])
                    xc.append(xt)
                    ec.append(et)
                    off += pc
                if last:
                    # counts2 rides BEHIND the final preds piece: it is not
                    # read until the epilogue STT (~5us later), and queueing
                    # it last pulls the tail-gating preds byte earlier.
                    nc.sync.dma_start(cnt_sb[:], counts2.ap())
                if k == 0:
                    nc.sync.dma_start(mask_sb[:], mask.ap())
                    nc.vector.memset(ones[:], 1.0)
                    nc.vector.memset(lnbias[:], 1e-30)

                # exp per chunk on ACT; row-sum partial per chunk on DVE.
                rs = sm.tile([128, len(pieces)], f32, tag="rs",
                             name=f"rs_{k}")
                for q, pc in enumerate(pieces):
                    nc.scalar.activation(ec[q][:], xc[q][:], AF.Exp)
                    nc.vector.tensor_reduce(rs[:, q:q + 1], ec[q][:], AX, ADD)
                w = sm.tile([128, 1], f32, tag="w")
                nc.vector.tensor_reduce(w[:], rs[:], AX, ADD)
                nc.vector.reciprocal(w[:], w[:])
                lh = sm.tile([128, 32], bf16, tag="lh")
                nc.vector.tensor_scalar_mul(lh[:], mask_sb[:], w[:])

                # Map class-chunk j (512 cols) to its source exp tile/slice.
                def esrc(j):
                    off = 0
                    for q, pc in enumerate(pieces):
                        if off + pc > j * 512:
                            return ec[q][:, j * 512 - off:(j + 1) * 512 - off]
                        off += pc
                    raise AssertionError

                # In the last iteration, order matmuls bank-major so each
                # bank's Ln + count-reduce can start while later banks still
                # accumulate.
                jorder = (
                    list(range(NCH)) if not last
                    else [0, 4, 8, 12, 1, 5, 9, 13, 2, 6, 10, 14, 3, 7, 11, 15]
                )
                for jj, j in enumerate(jorder):
                    nc.tensor.matmul(
                        psum_b[j % 4][32 * (j // 4):32 * (j // 4) + 32, :],
                        lh[:],
                        esrc(j),
                        start=(k == 0),
                        stop=last,
                        # Explicit: the auto-infer path rejects col base 96.
                        tile_position=(0, 32 * (j // 4)),
                        # Four accumulation groups share each bank on disjoint
                        # partition ranges; the sim's group-region check can't
                        # see the partition split, but the pending-zero value
                        # semantics handle it correctly.
                        skip_group_check=True,
                    )
                    if last and jj % 4 == 3:
                        # Bank i = jj // 4 fully accumulated: Ln + fused
                        # count-multiply row-sum for that bank, overlapping
                        # the remaining banks' matmuls.
                        i = jj // 4
                        cs = 512 * i
                        if i < 3:
                            nc.scalar.activation(
                                logt[i][:],
                                psum_b[i][:],
                                AF.Ln,
                                scale=1.0 / T,
                                bias=lnbias[:],
                            )
                            nc.vector.scalar_tensor_tensor(
                                prod[i][:], logt[i][:], 1.0,
                                cnt_sb[:, cs:cs + 512], op0=MUL, op1=MUL,
                                accum_out=red5[:, i:i + 1],
                            )
                        else:
                            # Last bank in halves: the final STT starts on
                            # half 0 while ACT runs Ln on half 1, shaving
                            # the serial Ln->STT handoff at the very end.
                            for h in range(2):
                                hs = 256 * h
                                nc.scalar.activation(
                                    logt[3][:, hs:hs + 256],
                                    psum_b[3][:, hs:hs + 256],
                                    AF.Ln,
                                    scale=1.0 / T,
                                    bias=lnbias[:],
                                )
                                nc.vector.scalar_tensor_tensor(
                                    prod[3][:, hs:hs + 256],
                                    logt[3][:, hs:hs + 256], 1.0,
                                    cnt_sb[:, cs + hs:cs + hs + 256],
                                    op0=MUL, op1=MUL,
                                    accum_out=red5[:, 3 + h:4 + h],
                                )

            # Collapse the 128x5 partials to five scalars with a ones-matmul
            # so the result DMA is a single 20-byte descriptor. Host sums.
            nc.tensor.matmul(psum_f[0:1, 0:5], ones[:], red5[:],
                             start=True, stop=True, skip_group_check=True)
            final = fin.tile([1, 5], f32, tag="final")
            nc.vector.tensor_copy(final[:], psum_f[0:1, 0:5])
            # Issue from the Sync queue (idle at the end, HWDGE fast path).
            nc.sync.dma_start(out.ap(), final[:])

    nc.compile()
    return nc


def _get_nc():
    if "nc" not in _CACHE:
        _CACHE["nc"] = _build_nc()
    return _CACHE["nc"]


def _host_counts(trans_targets: np.ndarray) -> np.ndarray:
    """Dense [B, C] float32 histogram with the blank-bin overwrite."""
    tt = np.asarray(trans_targets)
    valid = tt < C
    idx = np.where(valid, tt, C).astype(np.int64)
    counts = np.zeros((B, C + 1), np.float32)
    np.add.at(counts, (np.arange(B)[:, None], idx), 1.0)
    counts = counts[:, :C]
    ptl = np.sum(valid & (tt > 0), axis=1)
    counts[:, BLANK] = np.float32(T) - ptl.astype(np.float32)
    return counts


def _make_in_maps(preds: np.ndarray, trans_targets: np.ndarray):
    counts = _host_counts(trans_targets)
    mask = np.zeros((128, 32), np.float32)
    mask[np.arange(128), np.arange(128) % BL] = 1.0
    in_maps = []
    for i in range(NCORES):
        shard = np.ascontiguousarray(
            preds[:, i * BL:(i + 1) * BL, :], dtype=np.float32
        ).reshape(T * BL, C)
        cc = counts[i * BL:(i + 1) * BL]                  # [16, 8192]
        # chunk j -> partition 32*(j//4)+b, cols (j%4)*512+n
        c4 = cc.reshape(BL, 4, 4, 512)                    # [b, pg, bank, n]
        c2 = np.zeros((128, 2048), np.float32)
        for pg in range(4):
            c2[32 * pg:32 * pg + BL] = c4[:, pg].reshape(BL, 2048)
        in_maps.append({
            "preds": shard,
            "mask": mask,
            "counts2": c2.astype(ml_dtypes.bfloat16),
        })
    return in_maps


def kernel(preds: np.ndarray, trans_targets: np.ndarray) -> np.ndarray:
    from concourse.bass_utils import run_bass_kernel_spmd

    nc = _get_nc()
    in_maps = _make_in_maps(np.asarray(preds), np.asarray(trans_targets))
    res = run_bass_kernel_spmd(
        nc, in_maps, core_ids=list(range(NCORES)),
        trace=bool(_CACHE.get("trace", False)),
    )
    _CACHE["last_exec_ns"] = res.exec_time_ns
    _CACHE["last_res"] = res
    total = np.float64(0.0)
    for core_out in res.results:
        total += np.float64(core_out["out"].sum())
    loss = -total / (np.float64(B) * np.float64(T))
    return np.float32(loss)


# revision 12
# speedup vs baseline: 1.0253x; 1.0253x over previous
"""ACE loss kernel for Trainium2, data-parallel over batch across 8 NeuronCores.

Math (matches the reference exactly):
  p[b,c]   = mean_t softmax(preds[t,b,:])[c]
  counts   = per-row histogram of trans_targets (blank bin overwritten with
             T - #positive-valid labels); note sum_c counts[b,c] == T.
  loss     = -sum_{b,c} log(p[b,c]) * counts[b,c] / (B*T)

Device work per core (B_local = 16 batch rows, 64 MB of preds):
  One streaming pass over preds as 1MiB chunk DMAs ([128 partitions =
  (8 t x 16 b), 2048 classes] each; the final row-tile tapers to
  [2048,2048,2048,1024,512,512] so only a 512-col exp trails the last DMA).
  Each chunk lands in its OWN tile so its ACT exp gates only on its own
  DMA; row-sum partials run on the otherwise-idle DVE (bf16 2x reduce),
  keeping ACT's (N+352)/1.2ns exp stream just below the 432 GB/s DMA rate
  with no accumulator-read overhead -- in fast reps ACT otherwise overhangs
  the stream end by ~6us.
    ACT: E = exp(x) (bf16) per chunk
    DVE: rs[:,c] = rowsum(E chunk); w = 1/sum(rs); lhsT = mask32 * w
    PE : per 512-class chunk j, psum[32*(j//4)+0:32, (j%4)*512 ...] +=
         lhsT.T @ E   (PSUM accumulates the whole T reduction; the 16 zero
         lhsT columns write exact 0.0 rows, so all 128 psum partitions are
         deterministic and the epilogue can run full-width)
  Epilogue (pipelined bank-by-bank with the last tile's matmuls):
    4x ACT Ln(psum/T + 1e-30) -> bf16 over [128, 512]  (1e-30 keeps the
    zero rows finite: Ln -> -69, then * count 0 -> 0), fused DVE 16-bit
    multiply-by-counts (bf16 counts, exact for integers) + row-sum per
    bank (last bank in halves to pipeline ACT/DVE), ones-matmul collapse
    to five scalars, single 20-byte output DMA from the Sync queue.
Host: tiny histogram of trans_targets, final sum of the 8 per-core outputs.
"""

import sys

sys.path.insert(0, "/opt/trn_rl_repo")

import numpy as np
import ml_dtypes

T, B, C, L = 128, 128, 8192, 50
NCORES = 8
BL = B // NCORES          # 16 batch rows per core
NCH = C // 512            # 16 class chunks of 512
NT = (T * BL) // 128      # 16 row-tiles of 128 (t,b) pairs
BLANK = 0
# Per-tile DMA/exp chunking (512-aligned so every matmul rhs stays in one
# tile). Whole 4MiB contiguous DMAs stream fastest; the suffix tiles split
# so ACT's exp chain (at (N+352)/1.2ns + ~185ns/accum-read) never overhangs
# the stream end by more than ~receipt + one small chunk.
def _tile_pieces(k):
    if k <= 12:
        return (8192,)
    if k <= 14:
        return (4096, 4096)
    return (2048, 2048, 2048, 2048)

_CACHE = {}


def _build_nc():
    from concourse import bacc, mybir
    import concourse.tile as tile

    f32 = mybir.dt.float32
    bf16 = mybir.dt.bfloat16
    AF = mybir.ActivationFunctionType
    MUL = mybir.AluOpType.mult
    ADD = mybir.AluOpType.add
    AX = mybir.AxisListType.X

    nc = bacc.Bacc("TRN2", target_bir_lowering=False, debug=False)
    preds = nc.dram_tensor("preds", [T * BL, C], f32, kind="ExternalInput")
    mask = nc.dram_tensor("mask", [128, 32], f32, kind="ExternalInput")
    counts2 = nc.dram_tensor("counts2", [128, 2048], bf16, kind="ExternalInput")
    out = nc.dram_tensor("out", [1, 5], f32, kind="ExternalOutput")

    with tile.TileContext(nc) as tc:
        with tc.tile_pool(name="xp", bufs=3) as xp, \
             tc.tile_pool(name="xtp", bufs=1) as xtp, \
             tc.tile_pool(name="ep", bufs=3) as ep, \
             tc.tile_pool(name="etp", bufs=1) as etp, \
             tc.tile_pool(name="sm", bufs=2) as sm, \
             tc.tile_pool(name="fin", bufs=1) as fin, \
             tc.tile_pool(name="pp", bufs=1, space="PSUM") as pp:
            # xp/ep ring slots (sized by the largest tag use, 8192 cols)
            # carry tiles 0-14 (whole tiles + t13/t14 halves); the final
            # tile's quarters get dedicated xtp/etp tags so the ring WAR
            # waits can never stall the tail-critical DMAs.
            # Preload the ACT table set that holds BOTH Exp and Ln
            # (natural_log_exp_and_others, id 6) so no table switch lands on
            # the critical tail between the last exp and the Ln epilogue.
            nc.scalar.add_instruction(mybir.InstLoadActFuncSet(
                name=nc.get_next_instruction_name(), ins=[], outs=[],
                act_func_set_id=6))

            # p lives in psum banks 0-3: chunk j at partitions
            # [32*(j//4), +32) (rows 16-31 exact zeros), bank j % 4.
            # One tile PER BANK: the Tile framework tracks hazards at tile
            # granularity, so a shared tile serializes bank i+1's matmuls
            # behind bank i's Ln read (WAR). Separate tiles pipeline.
            psum_b = [pp.tile([128, 512], f32, tag=f"psum{i}", name=f"psum{i}")
                      for i in range(4)]
            # Bank 4: final ones-matmul collapse target.
            psum_f = pp.tile([128, 512], f32, tag="psumf", name="psum_f")

            # Small inputs + epilogue tiles (memset runs early, off the
            # DMA-critical path).
            mask_sb = fin.tile([128, 32], f32, tag="mask")
            cnt_sb = fin.tile([128, 2048], bf16, tag="cnt")
            # bf16 log tiles: halves DVE time in the count-multiply STT
            # (16-bit 2x mode); bf16 abs err ~0.03 on real rows -> ~0.1%
            # loss error, far inside tolerance.
            logt = [fin.tile([128, 512], bf16, tag=f"logt{i}", name=f"logt{i}")
                    for i in range(4)]
            prod = [fin.tile([128, 512], bf16, tag=f"prod{i}", name=f"prod{i}")
                    for i in range(4)]
            ones = fin.tile([128, 1], f32, tag="ones")
            lnbias = fin.tile([128, 1], f32, tag="lnbias")
            red5 = fin.tile([128, 5], f32, tag="red5")

            for k in range(NT):
                last = k == NT - 1
                rows = preds.ap()[k * 128:(k + 1) * 128, :]
                pieces = _tile_pieces(k)
                # One tile per chunk: exp gates on its own chunk's DMA.
                xc, ec = [], []
                off = 0
                for q, pc in enumerate(pieces):
                    if last:
                        xt = xtp.tile([128, pc], f32, tag=f"xt{q}",
                                      name=f"xt{q}")
                        et = etp.tile([128, pc], bf16, tag=f"et{q}",
                                      name=f"et{q}")
                    else:
                        xt = xp.tile([128, pc], f32, tag="xw",
                                     name=f"x{k}_{q}")
                        et = ep.tile([128, pc], bf16, tag="ew",
                                     name=f"e{k}_{q}")
                    nc.sync.dma_start(xt[:], rows[:, off:off + pc])
                    xc.append(xt)
                    ec.append(et)
                    off += pc
                if last:
                    # counts2 rides BEHIND the final preds piece: it is not
                    # read until the epilogue STT (~5us later), and queueing
                    # it last pulls the tail-gating preds byte earlier.
                    nc.sync.dma_start(cnt_sb[:], counts2.ap())
                if k == 0:
                    nc.sync.dma_start(mask_sb[:], mask.ap())
                    nc.vector.memset(ones[:], 1.0)
                    nc.vector.memset(lnbias[:], 1e-30)

                # exp per chunk on ACT, row-sum partial via the ACT
                # accumulator (reads cost ~185ns exposed, far cheaper than
                # a DVE reduce of the same columns).
                w = sm.tile([128, 1], f32, tag="w")
                if len(pieces) == 1:
                    nc.scalar.activation(ec[0][:], xc[0][:], AF.Exp,
                                         accum_out=w[:])
                else:
                    rs = sm.tile([128, len(pieces)], f32, tag="rs",
                                 name=f"rs_{k}")
                    for q, pc in enumerate(pieces):
                        nc.scalar.activation(ec[q][:], xc[q][:], AF.Exp,
                                             accum_out=rs[:, q:q + 1])
                    nc.vector.tensor_reduce(w[:], rs[:], AX, ADD)
                nc.vector.reciprocal(w[:], w[:])
                lh = sm.tile([128, 32], bf16, tag="lh")
                nc.vector.tensor_scalar_mul(lh[:], mask_sb[:], w[:])

                # Map class-chunk j (512 cols) to its source exp tile/slice.
                def esrc(j):
                    off = 0
                    for q, pc in enumerate(pieces):
                        if off + pc > j * 512:
                            return ec[q][:, j * 512 - off:(j + 1) * 512 - off]
                        off += pc
                    raise AssertionError

                # In the last iteration, order matmuls bank-major so each
                # bank's Ln + count-reduce can start while later banks still
                # accumulate.
                jorder = (
                    list(range(NCH)) if not last
                    else [0, 4, 8, 12, 1, 5, 9, 13, 2, 6, 10, 14, 3, 7, 11, 15]
                )
                for jj, j in enumerate(jorder):
                    nc.tensor.matmul(
                        psum_b[j % 4][32 * (j // 4):32 * (j // 4) + 32, :],
                        lh[:],
                        esrc(j),
                        start=(k == 0),
                        stop=last,
                        # Explicit: the auto-infer path rejects col base 96.
                        tile_position=(0, 32 * (j // 4)),
                        # Four accumulation groups share each bank on disjoint
                        # partition ranges; the sim's group-region check can't
                        # see the partition split, but the pending-zero value
                        # semantics handle it correctly.
                        skip_group_check=True,
                    )
                    if last and jj % 4 == 3:
                        # Bank i = jj // 4 fully accumulated: Ln + fused
                        # count-multiply row-sum for that bank, overlapping
                        # the remaining banks' matmuls.
                        i = jj // 4
                        cs = 512 * i
                        if i < 3:
                            nc.scalar.activation(
                                logt[i][:],
                                psum_b[i][:],
                                AF.Ln,
                                scale=1.0 / T,
                                bias=lnbias[:],
                            )
                            nc.vector.scalar_tensor_tensor(
                                prod[i][:], logt[i][:], 1.0,
                                cnt_sb[:, cs:cs + 512], op0=MUL, op1=MUL,
                                accum_out=red5[:, i:i + 1],
                            )
                        else:
                            # Last bank in halves: the final STT starts on
                            # half 0 while ACT runs Ln on half 1, shaving
                            # the serial Ln->STT handoff at the very end.
                            for h in range(2):
                                hs = 256 * h
                                nc.scalar.activation(
                                    logt[3][:, hs:hs + 256],
                                    psum_b[3][:, hs:hs + 256],
                                    AF.Ln,
                                    scale=1.0 / T,
                                    bias=lnbias[:],
                                )
                                nc.vector.scalar_tensor_tensor(
                                    prod[3][:, hs:hs + 256],
                                    logt[3][:, hs:hs + 256], 1.0,
                                    cnt_sb[:, cs + hs:cs + hs + 256],
                                    op0=MUL, op1=MUL,
                                    accum_out=red5[:, 3 + h:4 + h],
                                )

            # Collapse the 128x5 partials to five scalars with a ones-matmul
            # so the result DMA is a single 20-byte descriptor. Host sums.
            nc.tensor.matmul(psum_f[0:1, 0:5], ones[:], red5[:],
                             start=True, stop=True, skip_group_check=True)
            final = fin.tile([1, 5], f32, tag="final")
            nc.vector.tensor_copy(final[:], psum_f[0:1, 0:5])
            # Issue from the Sync queue (idle at the end, HWDGE fast path).
            nc.sync.dma_start(out.ap(), final[:])

    nc.compile()
    return nc


def _get_nc():
    if "nc" not in _CACHE:
        _CACHE["nc"] = _build_nc()
    return _CACHE["nc"]


def _host_counts(trans_targets: np.ndarray) -> np.ndarray:
    """Dense [B, C] float32 histogram with the blank-bin overwrite."""
    tt = np.asarray(trans_targets)
    valid = tt < C
    idx = np.where(valid, tt, C).astype(np.int64)
    counts = np.zeros((B, C + 1), np.float32)
    np.add.at(counts, (np.arange(B)[:, None], idx), 1.0)
    counts = counts[:, :C]
    ptl = np.sum(valid & (tt > 0), axis=1)
    counts[:, BLANK] = np.float32(T) - ptl.astype(np.float32)
    return counts


def _make_in_maps(preds: np.ndarray, trans_targets: np.ndarray):
    counts = _host_counts(trans_targets)
    mask = np.zeros((128, 32), np.float32)
    mask[np.arange(128), np.arange(128) % BL] = 1.0
    in_maps = []
    for i in range(NCORES):
        shard = np.ascontiguousarray(
            preds[:, i * BL:(i + 1) * BL, :], dtype=np.float32
        ).reshape(T * BL, C)
        cc = counts[i * BL:(i + 1) * BL]                  # [16, 8192]
        # chunk j -> partition 32*(j//4)+b, cols (j%4)*512+n
        c4 = cc.reshape(BL, 4, 4, 512)                    # [b, pg, bank, n]
        c2 = np.zeros((128, 2048), np.float32)
        for pg in range(4):
            c2[32 * pg:32 * pg + BL] = c4[:, pg].reshape(BL, 2048)
        in_maps.append({
            "preds": shard,
            "mask": mask,
            "counts2": c2.astype(ml_dtypes.bfloat16),
        })
    return in_maps


def kernel(preds: np.ndarray, trans_targets: np.ndarray) -> np.ndarray:
    from concourse.bass_utils import run_bass_kernel_spmd

    nc = _get_nc()
    in_maps = _make_in_maps(np.asarray(preds), np.asarray(trans_targets))
    res = run_bass_kernel_spmd(
        nc, in_maps, core_ids=list(range(NCORES)),
        trace=bool(_CACHE.get("trace", False)),
    )
    _CACHE["last_exec_ns"] = res.exec_time_ns
    _CACHE["last_res"] = res
    total = np.float64(0.0)
    for core_out in res.results:
        total += np.float64(core_out["out"].sum())
    loss = -total / (np.float64(B) * np.float64(T))
    return np.float32(loss)
